# revision 20
# baseline (speedup 1.0000x reference)
"""Trainium2 Bass kernel for nn_Code_Multiplexing — v7.4.

Per core (65536 batches):
  - 32768 batches: 2-stage complex butterfly in SBUF — 3 chunks:
    2x 12288 batches on DVE (fp16 2x-mode tensor_tensor, 4 wide ops each)
    and 1x 8192 batches on Pool.  No PE/PSUM involvement.
  - 32768 batches: fp16 matmul on PE (warmed) -> PSUM -> SBUF copies on
    ACT/DVE only (GPSIMD cannot access PSUM on trn2 hardware).
  - Loads spread SP/Pool (+2 pieces SP for xp); W first on SP.
  - All stores ride the 500ns descriptor floor (row-strided, 130-pad rows).
"""

import numpy as np

P = 128
B_FULL = 524288
N_CORES = 8
B_CORE = B_FULL // N_CORES      # 65536
FEAT = 32

QCS = [96, 96, 64]              # q-batches per butterfly chunk
NCHUNK = len(QCS)
CBS = [P * q for q in QCS]      # batches per chunk
NB = sum(CBS)                   # 32768
XPCS = [8 * 4 * q for q in QCS]     # xp cols per chunk (3072/3072/2048)
GCS = [4 * 4 * q for q in QCS]      # GA cols per chunk  (1536/1536/1024)
XPOFF = [sum(XPCS[:i]) for i in range(NCHUNK + 1)]
GOFF = [2 * sum(GCS[:i]) for i in range(NCHUNK + 1)]    # outab row offsets

BM = B_CORE - NB                # 32768 matmul batches
COLS = BM // 4                  # 8192 matmul columns
OPAD = 130

_CACHE = {}

BF_MAP_A = [[0, 8, 16, 24], [1, 9, 17, 25], [3, 11, 19, 27], [2, 10, 18, 26]]
BF_MAP_B = [[4, 12, 20, 28], [5, 13, 21, 29], [7, 15, 23, 31], [6, 14, 22, 30]]
BF_BLOCKS = [(0, 0), (1, 0), (1, 1), (0, 1), (2, 0), (3, 0), (3, 1), (2, 1)]

# ---- schedules (queue tags: S=sync A=scalar P=gpsimd) ----
BF_ENG = ["D", "D", "P"]        # butterfly compute engine per chunk
# xp load pieces: (chunk, piece-slice in 1024-col units, queue)
XP_LOADS = [(0, 0, "P"), (0, 1, "P"), (0, 2, "P"),
            (1, 0, "S"), (1, 1, "P"), (1, 2, "P"),
            (2, 0, "P"), (2, 1, "S")]
XQ = list("SSSSSSSS")           # x pieces cols 512..8192 all on SP
COPY_PLAN = list("AADAAAA")     # units 0-6; unit 7 is the tail (D/A split)
SPIECES = [(0, 4096, "P"), (4096, 7168, "S")]
TAIL_C0 = 7168
BF_STORE_Q = ["A", "S", "A"]
BF_AT_UNIT = {1: 0, 2: 2}       # chunk -> unit index to emit butterfly
BF_STORE_AT = {0: 2, 1: 4, 2: 6}


def _amatrix():
    Z = np.array(
        [[1, 1, 1, 1], [1j, -1j, 1j, -1j], [1, 1, -1, -1], [1j, -1j, -1j, 1j]],
        dtype=np.complex64,
    )
    A = np.zeros((FEAT, FEAT), np.float32)
    for o in range(4):
        for k in range(4):
            for j in range(4):
                re, im = float(Z[k, j].real), float(Z[k, j].imag)
                A[o * 8 + k * 2 + 0, j * 8 + o * 2 + 0] = re
                A[o * 8 + k * 2 + 0, j * 8 + o * 2 + 1] = -im
                A[o * 8 + k * 2 + 1, j * 8 + o * 2 + 0] = im
                A[o * 8 + k * 2 + 1, j * 8 + o * 2 + 1] = re
    return A


def _weight_matrix():
    A = _amatrix()
    W = np.zeros((P, P), np.float16)
    for blo in range(4):
        W[blo * 32:(blo + 1) * 32, blo * 32:(blo + 1) * 32] = A.T.astype(np.float16)
    return W


def _build_nc():
    import concourse.bacc as bacc
    import concourse.mybir as mybir
    from concourse.tile import TileContext

    f32 = mybir.dt.float32
    fp16 = mybir.dt.float16
    add = mybir.AluOpType.add
    sub = mybir.AluOpType.subtract
    nc = bacc.Bacc(None, target_bir_lowering=False)

    x = nc.dram_tensor("x", [P, COLS], fp16, kind="ExternalInput")
    xp = nc.dram_tensor("xp", [P, XPOFF[-1]], fp16, kind="ExternalInput")
    w = nc.dram_tensor("w", [P, P], fp16, kind="ExternalInput")
    out = nc.dram_tensor("out", [COLS, OPAD], fp16, kind="ExternalOutput")
    outab = nc.dram_tensor("outab", [GOFF[-1], OPAD], fp16,
                           kind="ExternalOutput")

    with TileContext(nc) as tc:
        with (
            tc.tile_pool(name="wpool", bufs=1) as wpool,
            tc.tile_pool(name="pool", bufs=1) as pool,
            tc.tile_pool(name="psum", bufs=4, space="PSUM") as psum_pool,
        ):
            X = pool.tile([P, COLS], fp16, name="x_t")
            G = pool.tile([P, COLS], fp16, name="g_t")
            Wt = wpool.tile([P, P], fp16, name="w_t")
            XPt = [pool.tile([P, XPCS[c]], fp16, name=f"xp{c}")
                   for c in range(NCHUNK)]
            T1t = [pool.tile([P, XPCS[c]], fp16, name=f"t1{c}")
                   for c in range(NCHUNK)]
            GABt = [pool.tile([P, 2 * GCS[c]], fp16, name=f"gab{c}")
                    for c in range(NCHUNK)]

            engs = {"S": nc.sync, "A": nc.scalar, "P": nc.gpsimd}
            veng = {"D": nc.vector, "P": nc.gpsimd}

            def xp_load(c, piece, q):
                c0 = piece * 1024
                c1 = min(c0 + 1024, XPCS[c])
                engs[q].dma_start(out=XPt[c][:, c0:c1],
                                  in_=xp[:, XPOFF[c] + c0:XPOFF[c] + c1])

            # phase 0: W + x0 on SP, then xp chunk loads per plan
            nc.sync.dma_start(out=Wt[:], in_=w[:])
            nc.sync.dma_start(out=X[:, :512], in_=x[:, :512])
            for (c, pc, q) in XP_LOADS[:3]:
                xp_load(c, pc, q)
            Wm = wpool.tile([P, 2], fp16, name="w_warm")
            nc.vector.memset(Wm[:], 0.0)
            ps0 = psum_pool.tile([P, 2], f32, tag="ps", name="ps_warm",
                                 padded_shape=[P, 1024])
            nc.tensor.matmul(ps0[:2, :2], Wm[:, :2], Wm[:, :2],
                             start=True, stop=True)
            for (c, pc, q) in XP_LOADS[3:]:
                xp_load(c, pc, q)

            def butterfly(c):
                e = veng[BF_ENG[c]]
                XPc, T1c = XPt[c], T1t[c]
                gc = GCS[c]
                xpb = XPc[:].rearrange("p (g k m) -> p g k m", g=2, k=4)
                in1 = xpb[:, :, 0::2]
                in2 = xpb[:, :, 1::2]
                t1b = T1c[:].rearrange("p (g k m) -> p g k m", g=2, k=4)
                e.tensor_tensor(t1b[:, :, 0:2], in1, in2, op=add)
                e.tensor_tensor(t1b[:, :, 2:4], in1, in2, op=sub)
                E = T1c[:, :gc]
                O = T1c[:, gc:]
                e.tensor_tensor(GABt[c][:, :gc], E, O, op=add)
                e.tensor_tensor(GABt[c][:, gc:], E, O, op=sub)

            def bf_store(c):
                engs[BF_STORE_Q[c]].dma_start(
                    out=outab[GOFF[c]:GOFF[c + 1], :128], in_=GABt[c][:])

            pieces = []
            col = 512
            for q in XQ:
                ln = min(1024, COLS - col)
                if ln <= 0:
                    break
                pieces.append((q, col, ln))
                col += ln
            assert col == COLS, (col, COLS)
            li = 0

            def emit_loads(upto):
                nonlocal li
                while li < len(pieces) and pieces[li][1] < upto:
                    q, c0, ln = pieces[li]
                    engs[q].dma_start(out=X[:, c0:c0 + ln], in_=x[:, c0:c0 + ln])
                    li += 1

            emit_loads(3 * 1024)
            butterfly(0)

            stores_done = 0

            def maybe_store(cdone):
                nonlocal stores_done
                while stores_done < len(SPIECES):
                    c0, c1, q = SPIECES[stores_done]
                    if cdone < c1:
                        break
                    engs[q].dma_start(out=out[c0:c1, :128], in_=G[:, c0:c1])
                    stores_done += 1

            nunits = (COLS + 1023) // 1024
            ucol = 0
            for ui in range(nunits):
                ln = min(1024, COLS - ucol)
                emit_loads(ucol + 3 * 1024)
                for (c, at) in BF_AT_UNIT.items():
                    if at == ui:
                        butterfly(c)
                for (c, at) in BF_STORE_AT.items():
                    if at == ui:
                        bf_store(c)
                ps = psum_pool.tile([P, ln], f32, tag="ps", name="ps",
                                    padded_shape=[P, 1024])
                for j in range(ln // 512):
                    cx = ucol + j * 512
                    nc.tensor.matmul(ps[:, j * 512:(j + 1) * 512],
                                     Wt[:], X[:, cx:cx + 512],
                                     start=True, stop=True,
                                     skip_group_check=True)
                if ui == nunits - 1:
                    h = ln // 2
                    nc.vector.tensor_copy(G[:, ucol:ucol + h], ps[:, :h])
                    nc.scalar.copy(G[:, ucol + h:ucol + ln], ps[:, h:ln])
                else:
                    if COPY_PLAN[ui] == "D":
                        nc.vector.tensor_copy(G[:, ucol:ucol + ln], ps[:])
                    else:
                        nc.scalar.copy(G[:, ucol:ucol + ln], ps[:])
                ucol += ln
                maybe_store(ucol if ui < nunits - 1 else TAIL_C0)
            nc.sync.dma_start(out=out[TAIL_C0:, :128], in_=G[:, TAIL_C0:])
    nc.compile()
    return nc


def _get_nc():
    if "nc" not in _CACHE:
        _CACHE["nc"] = _build_nc()
    return _CACHE["nc"]


def _pack_core(v):
    vm = v[:BM]
    xdev = (vm.reshape(COLS, 4, FEAT)
              .transpose(1, 2, 0)
              .reshape(P, COLS))
    xpd = np.empty((P, XPOFF[-1]), np.float16)
    boff = BM
    for c in range(NCHUNK):
        qc = QCS[c]
        vp = v[boff:boff + CBS[c]].reshape(qc, P, FEAT)     # [q, p, f]
        blk = np.empty((P, 8, 4, qc), np.float16)
        for bi, (j, r) in enumerate(BF_BLOCKS):
            bv = vp[:, :, j * 8 + np.arange(4) * 2 + r]     # [q, p, l]
            blk[:, bi] = bv.transpose(1, 2, 0)              # [p, l, q]
        xpd[:, XPOFF[c]:XPOFF[c + 1]] = blk.reshape(P, XPCS[c])
        boff += CBS[c]
    return np.ascontiguousarray(xdev), np.ascontiguousarray(xpd)


def _unpack_core(res):
    od = res["out"][:, :128]
    colmat = np.empty((COLS, P), od.dtype)
    bounds = [(c0, c1) for (c0, c1, _) in SPIECES] + [(TAIL_C0, COLS)]
    r0 = 0
    for (c0, c1) in bounds:
        n = c1 - c0
        Hp = od[r0:r0 + n].reshape(P, n // P, P).transpose(1, 2, 0)
        colmat[c0:c1] = Hp.reshape(n, P)
        r0 += n
    obm = colmat.reshape(COLS, 4, FEAT).reshape(BM, FEAT)

    yb = np.empty((NB, FEAT), od.dtype)
    oab = res["outab"][:, :128]
    boff = 0
    for c in range(NCHUNK):
        qc, gc = QCS[c], GCS[c]
        R = oab[GOFF[c]:GOFF[c + 1]]                # rows = (p, t)
        T = R.reshape(P, 2 * gc // P, P).reshape(P, 2 * gc)
        TA = T[:, :gc].reshape(P, 4, 4, qc)         # [p, g, l, q]
        TB = T[:, gc:].reshape(P, 4, 4, qc)
        for TT, mp in ((TA, BF_MAP_A), (TB, BF_MAP_B)):
            for g in range(4):
                for l in range(4):
                    yb[boff:boff + CBS[c], mp[g][l]] = (
                        TT[:, g, l, :].T.reshape(-1))
        boff += CBS[c]
    return np.concatenate([obm, yb], axis=0)


def kernel(x0, x1, x2, x3):
    from concourse.bass_utils import run_bass_kernel_spmd

    xs = [np.asarray(a, dtype=np.float32) for a in (x0, x1, x2, x3)]
    arr = np.stack(xs)                                  # [4j, B, 4l, 2r]
    W = _weight_matrix()
    nc = _get_nc()
    in_maps = []
    for c in range(N_CORES):
        sl = arr[:, c * B_CORE:(c + 1) * B_CORE]
        v = sl.transpose(1, 0, 2, 3).reshape(B_CORE, FEAT).astype(np.float16)
        xdev, xpd = _pack_core(v)
        in_maps.append({"x": xdev, "xp": xpd, "w": W})
    res = run_bass_kernel_spmd(nc, in_maps, core_ids=list(range(N_CORES))).results
    parts = [_unpack_core(res[c]) for c in range(N_CORES)]
    full = np.concatenate(parts, axis=0).astype(np.float32)
    full = full.reshape(B_FULL, 4, 4, 2)
    return tuple(np.ascontiguousarray(full[:, o]) for o in range(4))


# revision 23
# speedup vs baseline: 1.2121x; 1.2121x over previous
"""Trainium2 Bass kernel for nn_Code_Multiplexing — v6.

Math: per batch, a fixed 32x32 +/-1 map A over the 32 floats (4 streams x
4 l x re/im); y = A x. Per-core batch shard of 65536.

v6 = v5 (fp16 matmul path with batch-on-PSUM-partitions + cheap stores)
plus a Pool-engine butterfly side-channel: the last 8192 batches per core
are computed directly in SBUF with 10 fp16 tensor_tensor ops on the Pool
engine (2-stage radix-2 butterfly), bypassing PSUM entirely. That sheds
2048 columns from the PSUM->SBUF copy chain (DVE+ACT), which is the
binding resource, and uses Pool queue slack.
"""

import numpy as np

P = 128
B_FULL = 524288
N_CORES = 8
B_CORE = B_FULL // N_CORES      # 65536
FEAT = 32
M_B = 64                        # butterfly batches per partition
NB = P * M_B                    # 8192 butterfly batches per core
B_MAIN = B_CORE - NB            # 57344 matmul-path batches
COLS = B_MAIN // 4              # 14336 matmul columns (4 batches each)
OPAD = 130                      # padded row length of the main output
OPPAD = 33                      # padded row length of the butterfly output

_CACHE = {}


def _amatrix():
    Z = np.array(
        [[1, 1, 1, 1], [1j, -1j, 1j, -1j], [1, 1, -1, -1], [1j, -1j, -1j, 1j]],
        dtype=np.complex64,
    )
    A = np.zeros((FEAT, FEAT), np.float32)
    for o in range(4):
        for k in range(4):
            for j in range(4):
                re, im = float(Z[k, j].real), float(Z[k, j].imag)
                A[o * 8 + k * 2 + 0, j * 8 + o * 2 + 0] = re
                A[o * 8 + k * 2 + 0, j * 8 + o * 2 + 1] = -im
                A[o * 8 + k * 2 + 1, j * 8 + o * 2 + 0] = im
                A[o * 8 + k * 2 + 1, j * 8 + o * 2 + 1] = re
    return A


def _weight_matrix():
    # W[k = b_lo*32+f_in, n = b_lo*32+f_out] = A[f_out, f_in]
    A = _amatrix()
    W = np.zeros((P, P), np.float16)
    for blo in range(4):
        W[blo * 32:(blo + 1) * 32, blo * 32:(blo + 1) * 32] = A.T.astype(np.float16)
    return W


def _build_nc():
    import concourse.bacc as bacc
    import concourse.mybir as mybir
    from concourse.tile import TileContext

    f32 = mybir.dt.float32
    fp16 = mybir.dt.float16
    add = mybir.AluOpType.add
    sub = mybir.AluOpType.subtract
    nc = bacc.Bacc(None, target_bir_lowering=False)

    x = nc.dram_tensor("x", [P, COLS], fp16, kind="ExternalInput")
    xp = nc.dram_tensor("xp", [P, FEAT * M_B], fp16, kind="ExternalInput")
    w = nc.dram_tensor("w", [P, P], fp16, kind="ExternalInput")
    out = nc.dram_tensor("out", [COLS, OPAD], fp16, kind="ExternalOutput")
    outp = nc.dram_tensor("outp", [NB, OPPAD], fp16, kind="ExternalOutput")

    with TileContext(nc) as tc:
        with (
            tc.tile_pool(name="wpool", bufs=1) as wpool,
            tc.tile_pool(name="pool", bufs=1) as pool,
            tc.tile_pool(name="psum", bufs=4, space="PSUM") as psum_pool,
        ):
            X = pool.tile([P, COLS], fp16, name="x_t")
            XP = pool.tile([P, FEAT * M_B], fp16, name="xp_t")
            T1 = pool.tile([P, FEAT * M_B], fp16, name="t1")
            GP = pool.tile([P, FEAT * M_B], fp16, name="gp")
            G0 = pool.tile([P, COLS // 2], fp16, name="g0")
            G1 = pool.tile([P, COLS // 2], fp16, name="g1")
            Wt = wpool.tile([P, P], fp16, name="w_t")
            # first x pieces on SP/Pool; W rides the otherwise-idle ACT
            # HWDGE queue so it lands in parallel with them
            nc.sync.dma_start(out=X[:, :512], in_=x[:, :512])
            nc.scalar.dma_start(out=Wt[:], in_=w[:])
            # warm-up: start the PE p-state ramp clock and pull the ACT
            # activation-table load off the critical path, both during fill
            Wm = wpool.tile([P, 2], fp16, name="w_warm")
            nc.vector.memset(Wm[:], 0.0)
            Wm2 = wpool.tile([P, 2], fp16, name="w_warm2")
            nc.scalar.copy(Wm2[:], Wm[:])
            ps0 = psum_pool.tile([P, 2], f32, tag="ps", name="ps_warm",
                                 padded_shape=[P, 1024])
            nc.tensor.matmul(ps0[:2, :2], Wm[:, :2], Wm[:, :2],
                             start=True, stop=True)

            # remaining main loads; Pool also runs the butterfly + 2 stores,
            # so SP carries more pieces
            sp_pieces = [512, 512] + [1024] * 9
            pl_pieces = [512] + [1024] * 3
            assert sum(sp_pieces) + sum(pl_pieces) == COLS - 512
            col = 512
            pieces = []
            for i in range(max(len(sp_pieces), len(pl_pieces))):
                if i < len(sp_pieces):
                    pieces.append((nc.sync, sp_pieces[i]))
                if i < len(pl_pieces):
                    pieces.append((nc.gpsimd, pl_pieces[i]))
            npool = 0
            for eng, ln in pieces:
                eng.dma_start(out=X[:, col:col + ln], in_=x[:, col:col + ln])
                col += ln
                if eng is nc.gpsimd:
                    npool += 1
                    if npool == 1:
                        # butterfly input rides Pool after its first x piece
                        nc.gpsimd.dma_start(out=XP[:], in_=xp[:])
            assert col == COLS

            # ---- Pool butterfly over the last NB batches ----
            # XP feature blocks (4m wide each, layout (l, q) q-fastest):
            #   [a0, a1, a2, a3, b0, b1, b2, b3]  (a=re, b=im, index=stream j)
            # T1 blocks: [u1, u3, w1, v1, u2, u4, w2, v2]
            m4 = 4 * M_B

            def blk(t, i):
                return t[:, i * m4:(i + 1) * m4]

            s1 = [(0, 1, add, 0), (4, 5, add, 1), (5, 4, sub, 2), (0, 1, sub, 3),
                  (2, 3, add, 4), (6, 7, add, 5), (7, 6, sub, 6), (2, 3, sub, 7)]
            for i1, i2, op, o in s1:
                nc.gpsimd.tensor_tensor(blk(T1, o), blk(XP, i1), blk(XP, i2), op=op)
            # stage 2: [u1,u3,w1,v1] +/- [u2,u4,w2,v2] ->
            #   add: [reY0, imY0, reY1, imY1] -> f_out = l*8 + (0..3)
            #   sub: [reY2, imY2, reY3, imY3] -> f_out = l*8 + (4..7)
            in1 = T1[:, :4 * m4]
            in2 = T1[:, 4 * m4:]
            gp4 = GP[:].rearrange("p (q l c) -> p c l q", l=4, c=8)
            nc.gpsimd.tensor_tensor(gp4[:, 0:4], in1, in2, op=add)
            nc.gpsimd.tensor_tensor(gp4[:, 4:8], in1, in2, op=sub)
            # two halves: 6144 descriptors each stays under the loader's
            # static-ring limit
            nc.gpsimd.dma_start(out=outp[:NB // 2, :FEAT], in_=GP[:, :FEAT * M_B // 2])
            nc.gpsimd.dma_start(out=outp[NB // 2:, :FEAT], in_=GP[:, FEAT * M_B // 2:])

            # ---- matmul path: copy units (PSUM depth 4 at 1024) ----
            units = [512] * 4 + [1024] * 5 + [1024] * 5 + [512] * 4
            assert sum(units) == COLS
            busy = {"dve": 0.0, "act": -400.0}
            ucol = 0
            for ui, ln in enumerate(units):
                ps = psum_pool.tile([P, ln], f32, tag="ps", name="ps",
                                    padded_shape=[P, 1024])
                for i in range(ln // 128):
                    c = ucol // 128 + i
                    nc.tensor.matmul(ps[:, i * 128:(i + 1) * 128],
                                     X[:, c * 128:(c + 1) * 128], Wt[:],
                                     start=(i % 4 == 0), stop=(i % 4 == 3),
                                     skip_group_check=True)
                Gh, off = (G0, ucol) if ucol < COLS // 2 else (G1, ucol - COLS // 2)
                if ui == len(units) - 1:
                    # final unit: split across both engines for a short drain
                    h = ln // 2
                    nc.vector.tensor_copy(Gh[:, off:off + h], ps[:, :h])
                    nc.scalar.copy(Gh[:, off + h:off + ln], ps[:, h:])
                else:
                    cost_d = ln * 1.042 + 125
                    cost_a = ln * 0.833 + 185
                    if busy["dve"] + cost_d <= busy["act"] + cost_a:
                        busy["dve"] += cost_d
                        nc.vector.tensor_copy(Gh[:, off:off + ln], ps[:])
                    else:
                        busy["act"] += cost_a
                        nc.scalar.copy(Gh[:, off:off + ln], ps[:])
                ucol += ln

            # stores: plain [128, 7680] SBUF -> row-strided DRAM (130-elem
            # rows, 128 used); out AP free bytes 256 -> 500ns floor.
            # G0 completes mid-run (Pool queue fine); G1 is the tail store,
            # so it rides SP whose HWDGE init latency is ~170ns lower.
            nc.gpsimd.dma_start(out=out[:COLS // 2, :128], in_=G0[:])
            nc.sync.dma_start(out=out[COLS // 2:, :128], in_=G1[:])
    nc.compile()
    return nc


def _get_nc():
    if "nc" not in _CACHE:
        _CACHE["nc"] = _build_nc()
    return _CACHE["nc"]


def kernel(x0, x1, x2, x3):
    from concourse.bass_utils import run_bass_kernel_spmd

    xs = [np.asarray(a, dtype=np.float32) for a in (x0, x1, x2, x3)]
    arr = np.stack(xs)                                  # [4j, B, 4l, 2r]
    W = _weight_matrix()
    nc = _get_nc()
    in_maps = []
    for c in range(N_CORES):
        sl = arr[:, c * B_CORE:(c + 1) * B_CORE]        # [4j, B_CORE, 4, 2]
        # feature f = j*8 + l*2 + r ; main batch b -> (col=b//4, b_lo=b%4)
        v = sl.transpose(1, 0, 2, 3).reshape(B_CORE, FEAT).astype(np.float16)
        vm = v[:B_MAIN]
        xdev = (vm.reshape(COLS, 4, FEAT)               # [col, b_lo, f]
                 .transpose(1, 2, 0)                    # [b_lo, f, col]
                 .reshape(P, COLS))
        # butterfly batches: b_P -> (p = b_P%128, q = b_P//128)
        # XP[p, (r*4+j)*4m + l*m + q]   (f_in = j*8 + l*2 + r)
        vp = v[B_MAIN:].reshape(M_B, P, 4, 4, 2)        # [q, p, j, l, r]
        xpd = (vp.transpose(1, 4, 2, 3, 0)              # [p, r, j, l, q]
                 .reshape(P, FEAT * M_B))
        in_maps.append({"x": np.ascontiguousarray(xdev),
                        "xp": np.ascontiguousarray(xpd), "w": W})
    res = run_bass_kernel_spmd(nc, in_maps, core_ids=list(range(N_CORES))).results
    parts = []
    half = COLS // 2
    for c in range(N_CORES):
        od = res[c]["out"][:, :128]                     # [row, (b_lo, f_out)]
        colmat = np.empty((COLS, P), od.dtype)
        for h in range(2):
            H = od[h * half:(h + 1) * half]
            H = H.reshape(P, half // P, P).transpose(1, 0, 2)   # [a, p, b]
            colmat[h * half:(h + 1) * half] = H.reshape(half, P)
        obm = colmat.reshape(COLS, 4, FEAT).reshape(B_MAIN, FEAT)
        # butterfly rows: store order (p, q, f) -> batch = q*128 + p
        op_ = res[c]["outp"][:, :FEAT]                  # two (p, q-half) stores
        H0 = op_[:NB // 2].reshape(P, M_B // 2, FEAT)
        H1 = op_[NB // 2:].reshape(P, M_B // 2, FEAT)
        obp = (np.concatenate([H0, H1], axis=1)         # [p, q, f]
                 .transpose(1, 0, 2).reshape(NB, FEAT))
        parts.append(np.concatenate([obm, obp], axis=0))
    full = np.concatenate(parts, axis=0).astype(np.float32)     # [B, 32]
    full = full.reshape(B_FULL, 4, 4, 2)                # [b, o, k, r]
    return tuple(np.ascontiguousarray(full[:, o]) for o in range(4))

